# revision 1
# baseline (speedup 1.0000x reference)
"""GATv2 (2-layer, PyG semantics) on 8 Trainium2 NeuronCores — v2.

Restructure vs v1: per-edge work is reduced to (a) the z matmuls and
(b) ONE prefix-scan DVE op per 128-edge tile that yields all 8 head
logits (cumsum of sign*leaky(z~), head sums extracted via an
overlapping stride-0 output AP + one subtract).  Aggregation happens in
FEATURE space: y_h[d,f] = sum_e w_e*x[src_e,f] via a one-hot matmul with
the scaled 25-dim x rows, then one small matmul per block transforms
y -> num[d, 960].  relu(num)/denom is reduced against W2 columns with
two more scan ops.  One-hot tiles are precomputed on host and DMA'd.
Layer 2 reuses the per-block one-hot tiles kept resident in SBUF.
"""

import os
import sys

import numpy as np

if "/opt/trn_rl_repo" not in sys.path:
    sys.path.insert(0, "/opt/trn_rl_repo")

import ml_dtypes  # noqa: F401
from contextlib import ExitStack
from operator import add as _op_add

import concourse.bass as bass
import concourse.bacc as bacc
import concourse.tile as tile
from concourse import mybir
from concourse.bass_utils import run_bass_kernel_spmd

from concourse import dve_ops
from concourse.dve_spec import (
    Spec, Src0, Src1, C0, maxx, lower, _has_src1, scan, AluOp)
from concourse.dve_uop import DveOpSpec

LEAKY_OP_NAME = "LEAKY_MUL_REDUCE_GAT"
SCAN_OP_NAME = "LEAKY_MUL_SCAN_GAT"


def _leaky_ref(in0, in1, s0, s1, imm2):
    a = in0.astype(np.float32)
    b = (np.maximum(a, a * s0) * in1).astype(np.float32)
    return b, b.reshape(b.shape[0], -1).sum(axis=-1, keepdims=True)


def _scan_ref(in0, in1, s0, s1, imm2):
    a = in0.astype(np.float32)
    b = (np.maximum(a, a * s0) * in1).astype(np.float32)
    c = np.cumsum(b.reshape(b.shape[0], -1), axis=-1)
    return c.reshape(b.shape)


def _register(name, spec):
    if any(op.name == name for op in dve_ops.OPS):
        return next(op for op in dve_ops.OPS if op.name == name)
    shas = {}
    for ver in ("v3", "v4"):
        r = DveOpSpec(name=name, opcode=0, uops=lower(spec, ver=ver),
                      rd1_en=_has_src1(spec))
        shas[ver] = r.sha(ver)
    op = dve_ops.DveOp(name, spec, subdim=False, uops_sha=shas)
    dve_ops.OPS.append(op)
    dve_ops.CUSTOM_DVE_SPECS[op.name] = op.spec
    dve_ops._SUB_OPCODE_FOR_NAME[op.name] = (
        dve_ops._CUSTOM_DVE_ROW_BASE + len(dve_ops.OPS) - 1)
    assert dve_ops.get_dve_sub_opcode(op.name) < 0x20
    return op


LEAKY_OP = _register(
    LEAKY_OP_NAME,
    Spec(body=maxx(Src0, Src0 * C0) * Src1, accum=_op_add,
         accum_init=None, reference=_leaky_ref))
SCAN_OP = _register(
    SCAN_OP_NAME,
    Spec(body=scan(AluOp.ADD, maxx(Src0, Src0 * C0) * Src1),
         reference=_scan_ref))

F32 = mybir.dt.float32
F16 = mybir.dt.float16
I16 = mybir.dt.int16
NEG_SLOPE = 0.2


def _wrap16(idx, parts=128):
    n = idx.shape[0]
    assert n % 16 == 0
    w = np.asarray(idx, np.int16).reshape(n // 16, 16).T
    return np.tile(w, (parts // 16, 1))


class Prep:
    pass


def _prepare(x, edge_index, W1l, b1l, W1r, b1r, att1, bias1,
             W2l, b2l, W2r, b2r, att2, bias2, n_cores=8, T=12):
    p = Prep()
    N, F = x.shape
    H, C = att1.shape
    D = H * C
    E = edge_index.shape[1]
    assert N % n_cores == 0
    npc = N // n_cores
    FB = F + 1  # features + const col (25 incl const for F=23 -> 24)

    x = np.asarray(x, np.float32)
    src = np.concatenate([np.asarray(edge_index[0], np.int64), np.arange(N)])
    dst = np.concatenate([np.asarray(edge_index[1], np.int64), np.arange(N)])

    att_f = np.asarray(att1, np.float32).reshape(-1)
    u = np.maximum(np.abs(att_f), np.float32(1e-6))
    sign = (att_f / u).astype(np.float16)

    W1l_s = np.asarray(W1l, np.float32) * u[None, :]
    W1r_s = np.asarray(W1r, np.float32) * u[None, :]
    b1l_s = np.asarray(b1l, np.float32) * u
    b1r_s = np.asarray(b1r, np.float32) * u

    # x table: copy at rows 0 (src path) and offset 32 (dst path)
    x128 = np.zeros((N, 128), np.float32)
    x128[:, :F] = x
    x128[:, F] = 1.0
    x128[:, 32:32 + F] = x
    x128[:, 32 + F] = 1.0
    p.x128 = x128.astype(np.float16)

    w1l_t = np.zeros((128, D), np.float32)
    w1l_t[:F] = W1l_s
    w1l_t[F] = b1l_s
    p.w1l = w1l_t.astype(np.float16)
    w1r_t = np.zeros((128, D), np.float32)
    w1r_t[32:32 + F] = W1r_s
    w1r_t[32 + F] = b1r_s
    p.w1r = w1r_t.astype(np.float16)

    p.sign_tile = np.tile(sign[None, :], (128, 1)).astype(np.float16)

    # Wstack: chunk j rows 28*h'+f (h'=h-4j), cols = head h's 120 within half
    wst = np.zeros((2, 128, D // 2), np.float32)
    bias1_f = np.asarray(bias1, np.float32).reshape(H, C)
    b1l_f = np.asarray(b1l, np.float32).reshape(H, C)
    W1l_f = np.asarray(W1l, np.float32).reshape(F, H, C)
    for h in range(H):
        j, hp = divmod(h, 4)
        cols = slice(hp * C, (hp + 1) * C)
        wst[j, 28 * hp:28 * hp + F, cols] = W1l_f[:, h, :]
        wst[j, 28 * hp + F, cols] = b1l_f[h] + bias1_f[h]
    p.wst = wst.astype(np.float16)

    w2l_t = np.asarray(W2l, np.float32)[:, 0]
    w2r_t = np.asarray(W2r, np.float32)[:, 0]
    p.w2l_tile = np.tile(w2l_t[None, :], (128, 1)).astype(np.float16)
    p.w2r_tile = np.tile(w2r_t[None, :], (128, 1)).astype(np.float16)

    p.att2 = float(np.asarray(att2).reshape(-1)[0])
    p.b2l = float(np.asarray(b2l).reshape(-1)[0])
    p.b2r = float(np.asarray(b2r).reshape(-1)[0])
    p.bias2 = float(np.asarray(bias2).reshape(-1)[0])

    # ---- per-core block partition (greedy over dst-sorted edges) ----
    order = np.argsort(dst, kind="stable")
    src_s, dst_s = src[order], dst[order]
    deg = np.bincount(dst, minlength=N)
    cap = T * 128
    assert deg.max() <= cap, (deg.max(), cap)

    blocks = []
    for k in range(n_cores):
        blks, node = [], k * npc
        end = (k + 1) * npc
        while node < end:
            base, width, cnt = node, 0, 0
            while node < end and width < 128 and cnt + deg[node] <= cap:
                cnt += deg[node]
                width += 1
                node += 1
            assert width > 0
            blks.append((base, width))
        blocks.append(blks)
    B = max(len(b) for b in blocks)
    for blks in blocks:
        while len(blks) < B:
            blks.append((blks[0][0], 0))
    p.blocks, p.B, p.T, p.n_cores, p.N, p.D, p.H, p.Cd = (
        blocks, B, T, n_cores, N, D, H, C)
    p.npc = npc
    p.FB = FB
    nslot = n_cores * B * 128
    assert nslot < 32768, nslot

    node2slot = np.zeros(N, np.int64)
    for k in range(n_cores):
        for b, (base, width) in enumerate(blocks[k]):
            s0 = (k * B + b) * 128
            node2slot[base:base + width] = s0 + np.arange(width)

    edge_lo = np.searchsorted(dst_s, np.arange(0, N + 1, npc))
    p.in_maps = []
    for k in range(n_cores):
        es, ee = edge_lo[k], edge_lo[k + 1]
        ks, kd = src_s[es:ee], dst_s[es:ee]
        nreal = len([1 for (b0, w) in blocks[k] if w > 0])
        bounds = np.array([blocks[k][i][0] for i in range(nreal)] + [N + 1])
        kb = np.searchsorted(bounds, kd, side="right") - 1

        src_pad = np.zeros((B, cap), np.int64)
        dstl_pad = np.full((B, cap), -1, np.int64)
        sslot = np.zeros((B, cap), np.int64)
        dslot = np.zeros((B, cap), np.int64)
        for b in range(B):
            base, width = blocks[k][b]
            m = kb == b
            n = int(m.sum())
            assert n <= cap, (n, cap)
            if width == 0:
                assert n == 0
                continue
            src_pad[b, :n] = ks[m]
            dstl_pad[b, :n] = kd[m] - base
            sslot[b, :n] = node2slot[ks[m]]
            dslot[b, :n] = (kd[m] - base) + b * 128

        # one-hot tiles [B, T, 128 edges, 128 dst] f16 (pads -> all-zero row)
        dl = dstl_pad.reshape(B, T, 128)
        st = (dl[..., None] == np.arange(128)[None, None, None, :])
        st_tiles = st.astype(np.float16)
        stT_tiles = np.ascontiguousarray(st_tiles.transpose(0, 1, 3, 2))
        blk_ids = np.stack([
            np.minimum(np.arange(128) + blocks[k][b][0], N - 1)
            for b in range(B)])

        im = {
            "x128": p.x128,
            "w1l": p.w1l, "w1r": p.w1r,
            "wst": p.wst,
            "sign_tile": p.sign_tile,
            "w2l_tile": p.w2l_tile, "w2r_tile": p.w2r_tile,
            "st_tiles": st_tiles,
            "stT_tiles": stT_tiles,
            "xblkT": np.ascontiguousarray(
                p.x128[blk_ids].transpose(0, 2, 1)),
            "src_gidx": np.stack([_wrap16(src_pad[b]) for b in range(B)]),
            "l2_src": np.stack([_wrap16(sslot[b]) for b in range(B)]),
        }
        p.in_maps.append(im)
    return p


# ---------------------------------------------------------------------------
# Device program
# ---------------------------------------------------------------------------
def _build_program(p):
    n_cores, B, T, N, D, H = p.n_cores, p.B, p.T, p.N, p.D, p.H
    C = p.Cd
    cap = T * 128
    nslot = B * 128
    NH = D // 2  # 480
    FB = p.FB    # 24 (23 features + const)
    XW = 28      # per-head slot width in xs8
    CH = 4 * XW  # 112 chunk rows

    nc = bacc.Bacc("TRN2", target_bir_lowering=False, debug=False,
                   num_devices=n_cores)

    def din(name, shape, dt):
        return nc.dram_tensor(name, list(shape), dt, kind="ExternalInput").ap()

    x128 = din("x128", (N, 128), F16)
    w1l_d = din("w1l", (128, D), F16)
    w1r_d = din("w1r", (128, D), F16)
    wst_d = din("wst", (2, 128, NH), F16)
    sign_d = din("sign_tile", (128, D), F16)
    w2l_d = din("w2l_tile", (128, D), F16)
    w2r_d = din("w2r_tile", (128, D), F16)
    st_d = din("st_tiles", (B, T, 128, 128), F16)
    stT_d = din("stT_tiles", (B, T, 128, 128), F16)
    xblkT_d = din("xblkT", (B, 128, 128), F16)
    srcg_d = din("src_gidx", (B, 128, cap // 16), I16)
    l2s_d = din("l2_src", (B, 128, cap // 16), I16)

    cc_in = nc.dram_tensor("cc_in", [nslot, 64], F32).ap()
    cc_out = nc.dram_tensor("cc_out", [n_cores * nslot, 64], F32,
                            addr_space="Shared").ap()
    out2 = nc.dram_tensor("out2", [B, 128], F32, kind="ExternalOutput").ap()
    dbg_log = nc.dram_tensor("dbg_log", [128, 9], F32,
                             kind="ExternalOutput").ap()
    dbg_w8 = nc.dram_tensor("dbg_w8", [128, 8], F16,
                            kind="ExternalOutput").ap()
    dbg_xs8 = nc.dram_tensor("dbg_xs8", [128, 224], F16,
                             kind="ExternalOutput").ap()
    dbg_zsb = nc.dram_tensor("dbg_zsb", [128, 960], F16,
                             kind="ExternalOutput").ap()
    dbg_yt = nc.dram_tensor("dbg_yt", [128, 256], F16,
                            kind="ExternalOutput").ap()
    dbg_den = nc.dram_tensor("dbg_den", [128, 8], F32,
                             kind="ExternalOutput").ap()
    dbg_rl = nc.dram_tensor("dbg_rl", [128, 960], F16,
                            kind="ExternalOutput").ap()
    dbg_cc = nc.dram_tensor("dbg_cc", [128, 64], F32,
                            kind="ExternalOutput").ap()

    groups = [list(range(n_cores))]
    SUB = int(os.environ.get("GAT_L1SUB", "9"))
    TRICK = int(os.environ.get("GAT_TRICK", "1"))
    DBG = int(os.environ.get("GAT_DEBUG", "0"))
    stage = int(os.environ.get("GAT_STAGE", "3"))

    with tile.TileContext(nc) as tc, ExitStack() as ctx:
        cpool = ctx.enter_context(tc.tile_pool(name="consts", bufs=1))
        w1l_sb = cpool.tile([128, D], F16, tag="w1l")
        nc.sync.dma_start(w1l_sb[:], w1l_d[:])
        w1r_sb = cpool.tile([128, D], F16, tag="w1r")
        nc.sync.dma_start(w1r_sb[:], w1r_d[:])
        wst_sb = cpool.tile([128, 2, NH], F16, tag="wst")
        nc.sync.dma_start(wst_sb[:], wst_d[:].transpose((1, 0, 2)))
        sign_sb = cpool.tile([128, D], F16, tag="sg")
        nc.sync.dma_start(sign_sb[:], sign_d[:])
        w2l_sb = cpool.tile([128, D], F16, tag="w2l")
        nc.sync.dma_start(w2l_sb[:], w2l_d[:])
        w2r_sb = cpool.tile([128, D], F16, tag="w2r")
        nc.sync.dma_start(w2r_sb[:], w2r_d[:])
        ext9 = cpool.tile([128, 9], F32, tag="ext9")
        nc.vector.memset(ext9[:], 0.0)
        ext9b = cpool.tile([128, 9], F32, tag="ext9b")
        nc.vector.memset(ext9b[:], 0.0)
        scr8 = cpool.tile([128, 8], F32, tag="scr8")
        zero_sb = cpool.tile([128, 128], F16, tag="zero")
        nc.vector.memset(zero_sb[:], 0.0)

        gpool = ctx.enter_context(tc.tile_pool(name="gath", bufs=3))
        zspool = ctx.enter_context(tc.tile_pool(name="zsb", bufs=3))
        prefpool = ctx.enter_context(tc.tile_pool(name="pref", bufs=2))
        ipool = ctx.enter_context(tc.tile_pool(name="idx", bufs=3))
        stpool = ctx.enter_context(tc.tile_pool(name="st", bufs=3))
        xrpool = ctx.enter_context(tc.tile_pool(name="xr", bufs=2))
        spool = ctx.enter_context(tc.tile_pool(name="small", bufs=4))
        xspool = ctx.enter_context(tc.tile_pool(name="xs", bufs=3))
        ytpool = ctx.enter_context(tc.tile_pool(name="ytsb", bufs=2))
        rlpool = ctx.enter_context(tc.tile_pool(name="rl", bufs=2))
        ccpool = ctx.enter_context(tc.tile_pool(name="cc", bufs=2))
        chpool = ctx.enter_context(tc.tile_pool(name="cch", bufs=B))

        zctx = ExitStack()
        zpool = zctx.enter_context(tc.tile_pool(name="zp", bufs=2,
                                                space="PSUM"))
        ypool = zctx.enter_context(tc.tile_pool(name="yp", bufs=2,
                                                space="PSUM"))
        dpool = zctx.enter_context(tc.tile_pool(name="dp", bufs=2,
                                                space="PSUM"))

        cch_tiles = []
        GC = 512
        for b in range(B):
            # ---- per-block loads ----
            st_sb = stpool.tile([128, T, 128], F16, tag="st")
            nc.sync.dma_start(st_sb[:], st_d[b].transpose((1, 0, 2)))
            stT_sb = stpool.tile([128, T, 128], F16, tag="stT")
            nc.sync.dma_start(stT_sb[:], stT_d[b].transpose((1, 0, 2)))

            srcg_sb = ipool.tile([128, cap // 16], I16, tag="srcg")
            nc.sync.dma_start(srcg_sb[:], srcg_d[b])
            src_xT = gpool.tile([128, 1, cap], F16, tag="sx")
            for c in range(cap // GC):
                nc.gpsimd.dma_gather(
                    src_xT[:, :, bass.ts(c, GC)], x128[:, :],
                    srcg_sb[:, bass.ts(c, GC // 16)], GC, GC,
                    elem_size=128, transpose=True)
            # block's own 128 dst rows, host-gathered AND host-transposed:
            # a single plain DMA replaces the 27us gather call
            blk_xT = gpool.tile([128, 1, 128], F16, tag="blkx")
            nc.sync.dma_start(blk_xT[:, 0, :], xblkT_d[b])
            xr_ps = zpool.tile([128, 2, 512], F32, tag="z")
            for j in range(2):
                nc.tensor.matmul(xr_ps[:, j, 0:NH],
                                 lhsT=blk_xT[:, 0, :],
                                 rhs=w1r_sb[:, j * NH:(j + 1) * NH],
                                 start=True, stop=True)
            xr_sb = xrpool.tile([128, D], F16, tag="xr")
            nc.scalar.copy(xr_sb[:].rearrange("p (a b) -> p a b", a=2),
                           xr_ps[:, :, 0:NH])
            # normal-orientation gather of src rows (for scaled aggregation)
            src_n = gpool.tile([128, T, 128], F16, tag="sn")
            for c in range(cap // GC):
                nc.gpsimd.dma_gather(
                    src_n[:, c * (GC // 128):(c + 1) * (GC // 128), :],
                    x128[:, :], srcg_sb[:, bass.ts(c, GC // 16)], GC, GC,
                    elem_size=128, transpose=False)

            yT = ypool.tile([128, 512], F32, tag="yT")
            # full-bank dummy matmul: performs the has_written clear once,
            # ordered before both chunks via overlapping-write deps
            nc.tensor.matmul(yT[:, :], lhsT=zero_sb[:],
                             rhs=w1l_sb[:, 0:512], start=True, stop=True)
            den = dpool.tile([128, 8], F32, tag="den")

            for t in range(T if SUB >= 2 else 0):
                z = zpool.tile([128, 2, 512], F32, tag="z")
                for j in range(2):
                    nc.tensor.matmul(z[:, j, 0:NH],
                                     lhsT=src_xT[:, 0, bass.ts(t, 128)],
                                     rhs=w1l_sb[:, j * NH:(j + 1) * NH],
                                     start=True, stop=False)
                for j in range(2):
                    nc.tensor.matmul(z[:, j, 0:NH],
                                     lhsT=stT_sb[:, t, :],
                                     rhs=xr_sb[:, j * NH:(j + 1) * NH],
                                     start=False, stop=True)
                if SUB < 3:
                    continue
                # contiguous f16 copy of z (frees the PSUM banks early and
                # gives the scan a single uniform-stride input)
                z_sb = zspool.tile([128, D], F16, tag="zsb")
                nc.scalar.copy(z_sb[:].rearrange("p (a b) -> p a b", a=2),
                               z[:, :, 0:NH])
                # prefix-scan of sign*leaky(z~): head sums land in ext9[:,1:9]
                if TRICK:
                    nc.vector._custom_dve(
                        SCAN_OP,
                        out=ext9[:, 1:9].unsqueeze(2)
                            .broadcast_to([128, 8, C]),
                        in0=z_sb[:].rearrange("p (a b) -> p a b", a=8),
                        in1=sign_sb[:].rearrange("p (a b) -> p a b", a=8),
                        s0=NEG_SLOPE, s1=0.0, imm2=0.0)
                else:
                    prefix = prefpool.tile([128, D], F32, tag="pref")
                    nc.vector._custom_dve(
                        SCAN_OP, out=prefix[:],
                        in0=z_sb[:], in1=sign_sb[:],
                        s0=NEG_SLOPE, s1=0.0, imm2=0.0)
                    nc.vector.tensor_copy(
                        ext9[:, 1:9],
                        prefix[:].rearrange("p (a b) -> p a b", a=8)[:, :, C - 1])
                logits = spool.tile([128, 8], F32, tag="lg")
                nc.vector.scalar_tensor_tensor(
                    logits[:], ext9[:, 1:9], 0.0, ext9[:, 0:8],
                    op0=mybir.AluOpType.add,
                    op1=mybir.AluOpType.subtract)
                w8 = spool.tile([128, 8], F16, tag="ex")
                nc.scalar.activation(w8[:], logits[:],
                                     mybir.ActivationFunctionType.Exp)
                if DBG and b == 0 and t == 0:
                    nc.sync.dma_start(dbg_log[:, 0:8], logits[:])
                    nc.sync.dma_start(dbg_w8[:], w8[:])
                    nc.sync.dma_start(dbg_zsb[:], z_sb[:])
                if SUB < 4:
                    continue
                # xs8[e, h, f] = x[src_e, f] * w8[e, h]
                xs8 = xspool.tile([128, 8, XW], F16, tag="xs8")
                nc.vector.tensor_tensor(
                    xs8[:],
                    src_n[:, t, 0:XW].unsqueeze(1).broadcast_to([128, 8, XW]),
                    w8[:].unsqueeze(2).broadcast_to([128, 8, XW]),
                    op=mybir.AluOpType.mult)
                xs8f = xs8.rearrange("p a b -> p (a b)")
                if DBG and b == 0 and t == 0:
                    nc.sync.dma_start(dbg_xs8[:], xs8f[:])
                for j in range(2):
                    # one has_written chain per bank: only the very first
                    # matmul may clear the bank (start=True), else it wipes
                    # the other chunk's bits
                    nc.tensor.matmul(yT[0:CH, bass.ts(j, 128)],
                                     lhsT=xs8f[:, j * CH:(j + 1) * CH],
                                     rhs=st_sb[:, t, :],
                                     start=False,
                                     stop=(t == T - 1 and j == 1))
                nc.tensor.matmul(den[:, 0:8],
                                 lhsT=st_sb[:, t, :],
                                 rhs=w8[:],
                                 start=(t == 0), stop=(t == T - 1))

            # ---- block epilogue ----
            if SUB < 5:
                continue
            yT_sb = ytpool.tile([128, 256], F16, tag="ytsb")
            nc.scalar.copy(yT_sb[:], yT[:, 0:256])
            if DBG and b == 0:
                nc.sync.dma_start(dbg_yt[:], yT_sb[:])
            num = zpool.tile([128, 2, 512], F32, tag="z")
            for j in range(2):
                nc.tensor.matmul(num[:, j, 0:NH],
                                 lhsT=yT_sb[0:CH, bass.ts(j, 128)],
                                 rhs=wst_sb[0:CH, j, :],
                                 start=True, stop=True)
            dg = spool.tile([128, 8], F32, tag="dg")
            nc.vector.tensor_scalar_max(dg[:], den[:, 0:8], 1e-30)
            recipd = spool.tile([128, 8], F32, tag="rc")
            nc.vector.reciprocal(recipd[:], dg[:])
            if DBG and b == 0:
                nc.sync.dma_start(dbg_den[:], dg[:])
            if SUB < 6:
                continue
            relu_sb = rlpool.tile([128, D], F16, tag="rl")
            nc.scalar.activation(relu_sb[:].rearrange("p (a b) -> p a b", a=2),
                                 num[:, :, 0:NH],
                                 mybir.ActivationFunctionType.Relu)
            cc_sb = ccpool.tile([128, 64], F32, tag="cc")
            if SUB >= 7:
                phl = spool.tile([128, 8], F32, tag="phl")
                phr = spool.tile([128, 8], F32, tag="phr")
                for w2_sb, ph, e9 in ((w2l_sb, phl, ext9),
                                      (w2r_sb, phr, ext9b)):
                    if TRICK:
                        nc.vector._custom_dve(
                            SCAN_OP,
                            out=e9[:, 1:9].unsqueeze(2)
                                .broadcast_to([128, 8, C]),
                            in0=relu_sb[:].rearrange("p (a b) -> p a b", a=8),
                            in1=w2_sb[:].rearrange("p (a b) -> p a b", a=8),
                            s0=1.0, s1=0.0, imm2=0.0)
                    else:
                        prefix = prefpool.tile([128, D], F32, tag="pref")
                        nc.vector._custom_dve(
                            SCAN_OP, out=prefix[:],
                            in0=relu_sb[:], in1=w2_sb[:],
                            s0=1.0, s1=0.0, imm2=0.0)
                        nc.vector.tensor_copy(
                            e9[:, 1:9],
                            prefix[:].rearrange("p (a b) -> p a b",
                                                a=8)[:, :, C - 1])
                    nc.vector.scalar_tensor_tensor(
                        ph[:], e9[:, 1:9], 0.0, e9[:, 0:8],
                        op0=mybir.AluOpType.add,
                        op1=mybir.AluOpType.subtract)
                nc.vector._custom_dve(
                    LEAKY_OP, out=scr8[:], in0=phl[:], in1=recipd[:],
                    s0=1.0, s1=0.0, imm2=0.0, accum_out=cc_sb[:, 0:1])
                nc.vector._custom_dve(
                    LEAKY_OP, out=scr8[:], in0=phr[:], in1=recipd[:],
                    s0=1.0, s1=0.0, imm2=0.0, accum_out=cc_sb[:, 1:2])
            else:
                nc.vector.memset(cc_sb[:, 0:2], 0.0)
            if DBG and b == 0:
                nc.sync.dma_start(dbg_rl[:], relu_sb[:])
                nc.sync.dma_start(dbg_cc[:], cc_sb[:, :])
            nc.sync.dma_start(cc_in[bass.ts(b, 128), :], cc_sb[:, :])
            cch = chpool.tile([128, 2], F16, tag="cch")
            nc.vector.tensor_copy(cch[:], cc_sb[:, 0:2])
            cch_tiles.append(cch)

        zctx.close()

        # ---- collective: allgather the slot table ----
        if stage >= 2:
            tc.strict_bb_all_engine_barrier()
            nc.gpsimd.collective_compute(
                "AllGather", mybir.AluOpType.bypass, replica_groups=groups,
                ins=[cc_in[:, :]], outs=[cc_out[:, :]])
            tc.strict_bb_all_engine_barrier()

        # ---- layer 2 ----
        l2pool = ctx.enter_context(tc.tile_pool(name="l2", bufs=4))
        a2pool = ctx.enter_context(tc.tile_pool(name="agg2", bufs=3,
                                                space="PSUM"))
        for b in range(B if stage >= 3 else 0):
            l2s_sb = ipool.tile([128, cap // 16], I16, tag="srcg")
            nc.sync.dma_start(l2s_sb[:], l2s_d[b])
            gs = l2pool.tile([128, T, 64], F32, tag="gs")
            for c in range(cap // GC):
                nc.gpsimd.dma_gather(
                    gs[:, c * (GC // 128):(c + 1) * (GC // 128), :],
                    cc_out[:, :], l2s_sb[:, bass.ts(c, GC // 16)], GC, GC,
                    elem_size=64, transpose=False)
            st_sb = stpool.tile([128, T, 128], F16, tag="st")
            nc.sync.dma_start(st_sb[:], st_d[b].transpose((1, 0, 2)))
            stT_sb = stpool.tile([128, T, 128], F16, tag="stT")
            nc.sync.dma_start(stT_sb[:], stT_d[b].transpose((1, 0, 2)))
            # per-edge xr2[dst] via one-hot broadcast matmuls (no gather)
            gdp = a2pool.tile([128, 512], F32, tag="gdp")
            nc.tensor.matmul(gdp[:, :], lhsT=zero_sb[:],
                             rhs=w1l_sb[:, 0:512], start=True, stop=True)
            for t in range(T):
                nc.tensor.matmul(gdp[:, t:t + 1],
                                 lhsT=stT_sb[:, t, :],
                                 rhs=cch_tiles[b][:, 1:2],
                                 start=False, stop=(t == T - 1))

            # strided column extracts on ScalarE (idle engine), then all
            # DVE ops run on contiguous [128, T] tiles
            xl2e = l2pool.tile([128, T], F16, tag="xl2e")
            nc.scalar.add(xl2e[:], gs[:, :, 0], float(p.b2l))
            gd1 = l2pool.tile([128, T], F32, tag="gd1")
            nc.scalar.copy(gd1[:], gdp[:, 0:T])
            z2 = l2pool.tile([128, T], F32, tag="z2")
            nc.vector.scalar_tensor_tensor(
                z2[:], gd1[:], float(p.b2r), xl2e[:],
                op0=mybir.AluOpType.add, op1=mybir.AluOpType.add)
            lk = l2pool.tile([128, T], F32, tag="lk")
            nc.vector.scalar_tensor_tensor(
                lk[:], z2[:], NEG_SLOPE, z2[:],
                op0=mybir.AluOpType.mult, op1=mybir.AluOpType.max)
            ew2 = l2pool.tile([128, T], F16, tag="ew2")
            nc.scalar.activation(ew2[:], lk[:],
                                 mybir.ActivationFunctionType.Exp,
                                 scale=float(p.att2))
            rhs2 = l2pool.tile([128, 2, T], F16, tag="rhs2")
            nc.vector.tensor_tensor(rhs2[:, 0, :], ew2[:], xl2e[:],
                                    op=mybir.AluOpType.mult)
            nc.vector.tensor_copy(rhs2[:, 1, :], ew2[:])

            agg2 = a2pool.tile([128, 2], F32, tag="agg2")
            for t in range(T):
                nc.tensor.matmul(agg2[:, :], lhsT=st_sb[:, t, :],
                                 rhs=rhs2[:, :, t],
                                 start=(t == 0), stop=(t == T - 1))

            a2sb = spool.tile([128, 2], F32, tag="a2sb")
            nc.scalar.copy(a2sb[:], agg2[:])
            r2 = spool.tile([128, 1], F32, tag="r2")
            dn2 = spool.tile([128, 1], F32, tag="dn2")
            nc.vector.tensor_scalar_max(dn2[:], a2sb[:, 1:2], 1e-30)
            nc.vector.reciprocal(r2[:], dn2[:])
            o2 = spool.tile([128, 1], F32, tag="o2")
            nc.vector.tensor_scalar(o2[:], a2sb[:, 0:1], r2[:],
                                    float(p.bias2),
                                    op0=mybir.AluOpType.mult,
                                    op1=mybir.AluOpType.add)
            nc.sync.dma_start(out2[b, :], o2[:, 0])

    nc.compile()
    return nc


def kernel(x, edge_index, W1l, b1l, W1r, b1r, att1, bias1,
           W2l, b2l, W2r, b2r, att2, bias2, _trace=False):
    in_dt = np.asarray(x).dtype
    p = _prepare(np.asarray(x), np.asarray(edge_index),
                 np.asarray(W1l), np.asarray(b1l), np.asarray(W1r),
                 np.asarray(b1r), np.asarray(att1), np.asarray(bias1),
                 np.asarray(W2l), np.asarray(b2l), np.asarray(W2r),
                 np.asarray(b2r), np.asarray(att2), np.asarray(bias2))
    nc = _build_program(p)
    core_ids = list(range(p.n_cores))
    res = run_bass_kernel_spmd(nc, p.in_maps, core_ids, trace=_trace)
    out = np.zeros((p.N, 1), np.float32)
    for k in range(p.n_cores):
        o = res.results[k]["out2"]
        for b, (base, width) in enumerate(p.blocks[k]):
            if width:
                out[base:base + width, 0] = o[b, :width]
    kernel._last_results = res
    return out.astype(in_dt if np.issubdtype(in_dt, np.floating) else np.float32)

